# revision 1
# baseline (speedup 1.0000x reference)
"""Trainium2 Bass kernel: monomials x^a y^b z^c (a+b+c <= 3) for N=2M points.

Data-parallel across 8 NeuronCores; each core gets N/8 = 250k points padded
to 128*1960. Host assembles the trivial columns (1, x, y, z); the device
computes the 16 degree>=2 monomials. Inputs ship fp16 scaled 2^12, outputs
bf16 (scale-carrying; host applies exact power-of-two descales).

v3 design. Trace findings this session: exec_time = [~7.0us into the
preamble] .. [last engine retire + ~8.3us fixed sem-file-reset postamble].
The controllable part is the user window, and in v0 it was COMPUTE-paced:
DVE 18.6us + ACT 10.7us produced the 8MB of output at ~363 GB/s while the
DMA ring could sustain 410-456. Changes vs v0:
  1. Plane order (x2,y2,z2, xy,xz,yz, x3,xy2,xz2, x2y,x2z,xyz, y3,yz2,
     y2z,z3) makes the three squares one contiguous [P,3,f] ACTIVATE per
     tile (one 224-cyc fixed cost instead of three) and keeps every
     deg3 op a w-wide broadcast multiply: x*deg2[0:6], y*deg2[1:3],
     z*deg2[1:3]. DVE: 5 tensor_muls/tile, 13 planes. ACT: 1 square/tile,
     nothing else - DVE's s_q wait is never blocked (measured: DVE stream
     runs gap-free, last op 4us earlier than v0).
  2. Inputs as 4 tile-aligned group-DMAs (vs 7) - earlier arrival, fewer
     0.65us SP issue slots.
  3. All DMAs stay on the single SP HWDGE ring: a second (ACT) ring makes
     SDMA engine 15 a straggler (+4us on the last packet; its AXI port
     apparently also serves that ring's descriptor reads). SP-only drains
     all 16 engines within 120ns of each other.

Layouts (host packs/unpacks; point n <-> (p, col) = (n // 1960, n % 1960)):
  input  DRAM: per GROUP g (tile-aligned col ranges, widths G_LIST)
               [128, 3, gw] planar C-order; one DMA per group.
  SBUF  itb : [P, 3*F_TOTAL] f16, group-planar (x|y|z each gw wide).
  SBUF  otb : [P, 16*F_TOTAL] bf16, tile-planar (16 planes, each f wide).
  output DRAM: per tile [128, 16, f] C-order.

Raw bass (no Tile): standalone wait_ge ops only. Input DMAs use per-group
sems (unambiguous 16-counts); out-DMA sems accumulate per ring and are
waited only at kernel end (16*7), where partial interleave can't fake
completion.
"""

import sys
from contextlib import ExitStack

if "/opt/trn_rl_repo" not in sys.path:
    sys.path.insert(0, "/opt/trn_rl_repo")

import numpy as np
import concourse.bass as bass
import concourse.mybir as mybir
from concourse.bass_utils import run_bass_kernel_spmd

P = 128
K = 20
KD = 16  # device-computed columns (degree >= 2)
N_TOTAL = 2_000_000
N_CORES = 8
N_CORE = N_TOTAL // N_CORES  # 250_000
F_TOTAL = 1960
F_LIST = [96, 488, 440, 400, 288, 152, 96]  # small lead, big middle, small tail
G_LIST = [96, 488, 840, 536]  # input groups: g1 = tile1 alone -> lands ~1us earlier
TILE_GROUP = [0, 1, 2, 2, 3, 3, 3]
N_PAD = P * F_TOTAL  # 250_880

AF = mybir.ActivationFunctionType
F32 = mybir.dt.float32
BF16 = mybir.dt.bfloat16
F16 = mybir.dt.float16
SCALE_IN = 4096.0
DESCALE2 = 1.0 / 16777216.0  # 2^-24, deg2 planes
DESCALE3 = DESCALE2 / SCALE_IN  # 2^-36, deg3 planes

# Device planes: x2,y2,z2, xy,xz,yz, x3,xy2,xz2, x2y,x2z,xyz, y3,yz2,y2z,z3.
# Reference col order (4..19): x2,xy,xz,y2,yz,z2, x3,x2y,x2z,xy2,xyz,xz2,
# y3,y2z,yz2,z3. COL_TO_PLANE[j] = device plane holding ref column 4+j.
COL_TO_PLANE = [0, 3, 4, 1, 5, 2, 6, 9, 10, 7, 11, 8, 12, 14, 13, 15]


def build(nc: bass.Bass, f_list, g_list, tile_group) -> bass.Bass:
    t_total = len(f_list)
    g_total = len(g_list)
    f_sum = sum(f_list)
    assert sum(g_list) == f_sum
    offs = np.concatenate([[0], np.cumsum(f_list)]).astype(int)
    goffs = np.concatenate([[0], np.cumsum(g_list)]).astype(int)

    v = nc.declare_dram_parameter("vectors", [P * 3 * f_sum], F16, isOutput=False)
    o = nc.declare_dram_parameter("out", [P * KD * f_sum], BF16, isOutput=True)

    with ExitStack() as ctx:
        itb = ctx.enter_context(nc.sbuf_tensor("itb", [P, 3 * f_sum], F16))
        otb = ctx.enter_context(nc.sbuf_tensor("otb", [P, KD * f_sum], BF16))
        s_in = [ctx.enter_context(nc.semaphore(f"s_in{g}")) for g in range(g_total)]
        s_q = ctx.enter_context(nc.semaphore("s_q"))  # ACT squares done
        s_x = ctx.enter_context(nc.semaphore("s_x"))  # DVE deg2 cross done
        s_dA = ctx.enter_context(nc.semaphore("s_dA"))  # DVE 6-wide deg3 done
        s_d3 = ctx.enter_context(nc.semaphore("s_d3"))  # DVE deg3 done
        s_os = ctx.enter_context(nc.semaphore("s_os"))
        block = ctx.enter_context(nc.Block(no_gpsimd_drain=True))

        def in3(t, k, w=1):
            """[P, w, f_t] view of input planes k..k+w for tile t."""
            g = tile_group[t]
            gw = g_list[g]
            r = int(offs[t] - goffs[g])
            f = f_list[t]
            return itb.ap()[:, 3 * goffs[g] : 3 * goffs[g + 1]].rearrange(
                "p (c g) -> p c g", g=gw
            )[:, k : k + w, r : r + f]

        def ot_flat(t):
            return otb.ap()[:, KD * offs[t] : KD * offs[t + 1]]

        def plane(t, k, w=1):
            f = f_list[t]
            return ot_flat(t)[:, k * f : (k + w) * f].rearrange(
                "p (c f) -> p c f", f=f
            )

        def bcast(t, k, w):
            return in3(t, k, 1).broadcast_to([P, w, f_list[t]])

        def od_flat(t):
            f = f_list[t]
            base = P * KD * offs[t]
            return o[base : base + P * KD * f].rearrange("(p q) -> p q", p=P)

        @block.sync
        def _(sync):
            # Single SP HWDGE ring for ALL DMAs: the second (ACT) ring makes
            # SDMA engine 15 a 12% straggler (its AXI port apparently also
            # serves that ring's descriptor reads; measured +4us on the last
            # packet), while the SP-only config drains all 16 engines within
            # 120ns of each other. FIFO order = readiness order: inputs,
            # then per tile deg2 (ready at cross+squares, early in DVE's
            # tile) followed by deg3 (ready at tile end).
            for g in range(g_total):
                vd = v[P * 3 * goffs[g] : P * 3 * goffs[g + 1]].rearrange(
                    "(p q) -> p q", p=P
                )
                sync.dma_start(
                    out=itb.ap()[:, 3 * goffs[g] : 3 * goffs[g + 1]], in_=vd
                ).then_inc(s_in[g], 16)
            n_inc = 0
            for t in range(t_total):
                f = f_list[t]
                if f <= 160:
                    # small tile: one merged [0:16] store (s_d3 implies s_x
                    # and s_dA by DVE program order; s_q covers ACT squares)
                    sync.wait_ge(s_q, t + 1)
                    sync.wait_ge(s_d3, t + 1)
                    sync.dma_start(
                        out=od_flat(t), in_=ot_flat(t)
                    ).then_inc(s_os, 16)
                    n_inc += 16
                    continue
                sync.wait_ge(s_q, t + 1)
                sync.wait_ge(s_x, t + 1)
                sync.dma_start(
                    out=od_flat(t)[:, 0 : 6 * f], in_=ot_flat(t)[:, 0 : 6 * f]
                ).then_inc(s_os, 16)
                if t <= 2:
                    # ramp tiles: fine 3-way split so deg3 bytes hit the
                    # still-thin ring queue as early as possible
                    sync.wait_ge(s_dA, t + 1)
                    sync.dma_start(
                        out=od_flat(t)[:, 6 * f : 12 * f],
                        in_=ot_flat(t)[:, 6 * f : 12 * f],
                    ).then_inc(s_os, 16)
                    sync.wait_ge(s_d3, t + 1)
                    sync.dma_start(
                        out=od_flat(t)[:, 12 * f : KD * f],
                        in_=ot_flat(t)[:, 12 * f : KD * f],
                    ).then_inc(s_os, 16)
                    n_inc += 48
                else:
                    # steady-state tiles: ring is backlogged, so chunk
                    # readiness is irrelevant - one big [6:16] transfer
                    # halves the per-DMA ring overhead
                    sync.wait_ge(s_d3, t + 1)
                    sync.dma_start(
                        out=od_flat(t)[:, 6 * f : KD * f],
                        in_=ot_flat(t)[:, 6 * f : KD * f],
                    ).then_inc(s_os, 16)
                    n_inc += 32
            sync.wait_ge(s_os, n_inc)

        @block.scalar
        def _(scalar):
            # ACT: one 3-wide square per tile (planes 0:3 = x2,y2,z2), fully
            # decoupled from the DMA rings so DVE's s_q wait never stalls.
            seen = set()
            for t in range(t_total):
                g = tile_group[t]
                if g not in seen:
                    seen.add(g)
                    scalar.wait_ge(s_in[g], 16)
                nc.scalar.square(plane(t, 0, 3), in3(t, 0, 3)).then_inc(s_q, 1)

        @block.vector
        def _(vector):
            seen = set()
            for t in range(t_total):
                g = tile_group[t]
                if g not in seen:
                    seen.add(g)
                    vector.wait_ge(s_in[g], 16)
                # deg2 cross: xy,xz -> planes 3:5; yz -> plane 5
                nc.vector.tensor_mul(plane(t, 3, 2), bcast(t, 0, 2), in3(t, 1, 2))
                nc.vector.tensor_mul(plane(t, 5), in3(t, 1), in3(t, 2)).then_inc(
                    s_x, 1
                )
                # deg3: x*deg2[0:6] -> 6:12 (x3,xy2,xz2,x2y,x2z,xyz);
                # y*deg2[1:3] -> 12:14 (y3,yz2); z*deg2[1:3] -> 14:16
                # (y2z,z3). Needs ACT's squares (planes 0:3).
                vector.wait_ge(s_q, t + 1)
                nc.vector.tensor_mul(
                    plane(t, 6, 6), bcast(t, 0, 6), plane(t, 0, 6)
                ).then_inc(s_dA, 1)
                nc.vector.tensor_mul(plane(t, 12, 2), bcast(t, 1, 2), plane(t, 1, 2))
                nc.vector.tensor_mul(
                    plane(t, 14, 2), bcast(t, 2, 2), plane(t, 1, 2)
                ).then_inc(s_d3, 1)

    return nc


_CACHE: dict[str, object] = {}


def _get_nc() -> bass.Bass:
    if "nc" not in _CACHE:
        nc = bass.Bass(enable_partition_id=False, monotonic_sem_count=0)
        build(nc, F_LIST, G_LIST, TILE_GROUP)
        _CACHE["nc"] = nc
    return _CACHE["nc"]  # type: ignore[return-value]


def run_spmd(in_maps, trace=False, **kw):
    return run_bass_kernel_spmd(
        _get_nc(), in_maps, core_ids=list(range(N_CORES)), trace=trace, **kw
    )


_GOFFS = np.concatenate([[0], np.cumsum(G_LIST)]).astype(int)
_OFFS = np.concatenate([[0], np.cumsum(F_LIST)]).astype(int)


def to_planar(shard: np.ndarray) -> np.ndarray:
    """[n_pad, 3] f32 -> flat [P*3*F_TOTAL] per-GROUP planar blocks."""
    arr = shard.reshape(P, F_TOTAL, 3)
    parts = [
        np.ascontiguousarray(
            arr[:, _GOFFS[g] : _GOFFS[g + 1], :].transpose(0, 2, 1)
        ).reshape(-1)
        for g in range(len(G_LIST))
    ]
    return np.concatenate(parts)


# descale in REFERENCE column order: deg2 cols 0-5, deg3 cols 6-15
_PLANE_DESCALE_REF = np.array([DESCALE2] * 6 + [DESCALE3] * 10, dtype=np.float32)
_COL_TO_PLANE = np.asarray(COL_TO_PLANE)


def from_planar(dev_out: np.ndarray) -> np.ndarray:
    """flat [P*16*F_TOTAL] (any dtype) -> [n_pad, 16] f32 in ref order."""
    arr = np.asarray(dev_out, dtype=np.float32).reshape(-1)
    out = np.empty((P, F_TOTAL, KD), dtype=np.float32)
    for t in range(len(F_LIST)):
        f = F_LIST[t]
        pos = P * KD * _OFFS[t]
        blk = arr[pos : pos + P * KD * f].reshape(P, KD, f).transpose(0, 2, 1)
        out[:, _OFFS[t] : _OFFS[t + 1], :] = blk[:, :, _COL_TO_PLANE]
    out *= _PLANE_DESCALE_REF
    return out.reshape(N_PAD, KD)


def make_in_maps(vectors: np.ndarray):
    vectors = np.ascontiguousarray(np.asarray(vectors, dtype=np.float32))
    assert vectors.shape == (N_TOTAL, 3)
    shards = vectors.reshape(N_CORES, N_CORE, 3)
    in_maps = []
    for i in range(N_CORES):
        buf = np.zeros((N_PAD, 3), dtype=np.float32)
        buf[:N_CORE] = shards[i]
        in_maps.append(
            {"vectors": (to_planar(buf) * np.float32(SCALE_IN)).astype(np.float16)}
        )
    return in_maps


def kernel(vectors: np.ndarray) -> np.ndarray:
    vec32 = np.ascontiguousarray(np.asarray(vectors, dtype=np.float32))
    res = run_spmd(make_in_maps(vec32))
    out = np.empty((N_TOTAL, K), dtype=np.float32)
    out[:, 0] = 1.0
    out[:, 1:4] = vec32  # degree-1 monomials are the input, exactly
    for i in range(N_CORES):
        out[i * N_CORE : (i + 1) * N_CORE, 4:] = from_planar(res.results[i]["out"])[
            :N_CORE
        ]
    return out



# revision 3
# speedup vs baseline: 1.4542x; 1.4542x over previous
"""Trainium2 Bass kernel: monomials x^a y^b z^c (a+b+c <= 3) for N=2M points.

Data-parallel across 8 NeuronCores; each core gets N/8 = 250k points padded
to 128*1960. Host assembles the trivial columns (1, x, y, z); the device
computes the 16 degree>=2 monomials. Inputs ship fp16 scaled 2^12, outputs
bf16 (scale-carrying; host applies exact power-of-two descales).

v3 design. Trace findings this session: exec_time = [~7.0us into the
preamble] .. [last engine retire + ~8.3us fixed sem-file-reset postamble].
The controllable part is the user window, and in v0 it was COMPUTE-paced:
DVE 18.6us + ACT 10.7us produced the 8MB of output at ~363 GB/s while the
DMA ring could sustain 410-456. Changes vs v0:
  1. Plane order (x2,y2,z2, xy,xz,yz, x3,xy2,xz2, x2y,x2z,xyz, y3,yz2,
     y2z,z3) makes the three squares one contiguous [P,3,f] ACTIVATE per
     tile (one 224-cyc fixed cost instead of three) and keeps every
     deg3 op a w-wide broadcast multiply: x*deg2[0:6], y*deg2[1:3],
     z*deg2[1:3]. DVE: 5 tensor_muls/tile, 13 planes. ACT: 1 square/tile,
     nothing else - DVE's s_q wait is never blocked (measured: DVE stream
     runs gap-free, last op 4us earlier than v0).
  2. Inputs as 4 tile-aligned group-DMAs (vs 7) - earlier arrival, fewer
     0.65us SP issue slots.
  3. All DMAs stay on the single SP HWDGE ring: a second (ACT) ring makes
     SDMA engine 15 a straggler (+4us on the last packet; its AXI port
     apparently also serves that ring's descriptor reads). SP-only drains
     all 16 engines within 120ns of each other.

Layouts (host packs/unpacks; point n <-> (p, col) = (n // 1960, n % 1960)):
  input  DRAM: per GROUP g (tile-aligned col ranges, widths G_LIST)
               [128, 3, gw] planar C-order; one DMA per group.
  SBUF  itb : [P, 3*F_TOTAL] f16, group-planar (x|y|z each gw wide).
  SBUF  otb : [P, 16*F_TOTAL] bf16, tile-planar (16 planes, each f wide).
  output DRAM: per tile [128, 16, f] C-order.

Raw bass (no Tile): standalone wait_ge ops only. Input DMAs use per-group
sems (unambiguous 16-counts); out-DMA sems accumulate per ring and are
waited only at kernel end (16*7), where partial interleave can't fake
completion.
"""

import sys
from contextlib import ExitStack

if "/opt/trn_rl_repo" not in sys.path:
    sys.path.insert(0, "/opt/trn_rl_repo")

import numpy as np
import concourse.bass as bass
import concourse.mybir as mybir
from concourse.bass_utils import run_bass_kernel_spmd

P = 128
K = 20
KD = 16  # device-computed columns (degree >= 2)
N_TOTAL = 2_000_000
N_CORES = 8
N_CORE = N_TOTAL // N_CORES  # 250_000
F_TOTAL = 1960
F_LIST = [96, 488, 440, 400, 288, 152, 96]  # small lead, big middle, small tail
G_LIST = [96, 488, 840, 536]  # input groups: g1 = tile1 alone -> lands ~1us earlier
TILE_GROUP = [0, 1, 2, 2, 3, 3, 3]
N_PAD = P * F_TOTAL  # 250_880

AF = mybir.ActivationFunctionType
F32 = mybir.dt.float32
BF16 = mybir.dt.bfloat16
F16 = mybir.dt.float16
SCALE_IN = 4096.0
DESCALE2 = 1.0 / 16777216.0  # 2^-24, deg2 planes
DESCALE3 = DESCALE2 / SCALE_IN  # 2^-36, deg3 planes

# Device planes: x2,y2,z2, xy,xz,yz, x3,xy2,xz2, x2y,x2z,xyz, y3,yz2,y2z,z3.
# Reference col order (4..19): x2,xy,xz,y2,yz,z2, x3,x2y,x2z,xy2,xyz,xz2,
# y3,y2z,yz2,z3. COL_TO_PLANE[j] = device plane holding ref column 4+j.
COL_TO_PLANE = [0, 3, 4, 1, 5, 2, 6, 9, 10, 7, 11, 8, 12, 14, 13, 15]


def build(nc: bass.Bass, f_list, g_list, tile_group) -> bass.Bass:
    t_total = len(f_list)
    g_total = len(g_list)
    f_sum = sum(f_list)
    assert sum(g_list) == f_sum
    offs = np.concatenate([[0], np.cumsum(f_list)]).astype(int)
    goffs = np.concatenate([[0], np.cumsum(g_list)]).astype(int)

    v = nc.declare_dram_parameter("vectors", [P * 3 * f_sum], F16, isOutput=False)
    o = nc.declare_dram_parameter("out", [P * KD * f_sum], BF16, isOutput=True)

    with ExitStack() as ctx:
        itb = ctx.enter_context(nc.sbuf_tensor("itb", [P, 3 * f_sum], F16))
        otb = ctx.enter_context(nc.sbuf_tensor("otb", [P, KD * f_sum], BF16))
        warm = ctx.enter_context(nc.sbuf_tensor("warm", [P, 2], BF16))
        s_in = [ctx.enter_context(nc.semaphore(f"s_in{g}")) for g in range(g_total)]
        s_q = ctx.enter_context(nc.semaphore("s_q"))  # ACT squares done
        s_x = ctx.enter_context(nc.semaphore("s_x"))  # DVE deg2 cross done
        s_dA = ctx.enter_context(nc.semaphore("s_dA"))  # DVE 6-wide deg3 done
        s_d3 = ctx.enter_context(nc.semaphore("s_d3"))  # DVE deg3 done
        s_os = ctx.enter_context(nc.semaphore("s_os"))
        block = ctx.enter_context(nc.Block(no_gpsimd_drain=True))

        def in3(t, k, w=1):
            """[P, w, f_t] view of input planes k..k+w for tile t."""
            g = tile_group[t]
            gw = g_list[g]
            r = int(offs[t] - goffs[g])
            f = f_list[t]
            return itb.ap()[:, 3 * goffs[g] : 3 * goffs[g + 1]].rearrange(
                "p (c g) -> p c g", g=gw
            )[:, k : k + w, r : r + f]

        def ot_flat(t):
            return otb.ap()[:, KD * offs[t] : KD * offs[t + 1]]

        def plane(t, k, w=1):
            f = f_list[t]
            return ot_flat(t)[:, k * f : (k + w) * f].rearrange(
                "p (c f) -> p c f", f=f
            )

        def bcast(t, k, w):
            return in3(t, k, 1).broadcast_to([P, w, f_list[t]])

        def od_flat(t):
            f = f_list[t]
            base = P * KD * offs[t]
            return o[base : base + P * KD * f].rearrange("(p q) -> p q", p=P)

        @block.sync
        def _(sync):
            # Single SP HWDGE ring for ALL DMAs: the second (ACT) ring makes
            # SDMA engine 15 a 12% straggler (its AXI port apparently also
            # serves that ring's descriptor reads; measured +4us on the last
            # packet), while the SP-only config drains all 16 engines within
            # 120ns of each other. FIFO order = readiness order: inputs,
            # then per tile deg2 (ready at cross+squares, early in DVE's
            # tile) followed by deg3 (ready at tile end).
            #
            # v4: no completion wait at the end. The NEFF epilogue (walrus's
            # full sem-file reset, ~8us, split across the 5 engines) runs as
            # soon as every engine retires its program; without a final
            # s_os wait it overlaps the in-flight DMA drain instead of
            # serializing after it. Data correctness is unaffected: the
            # packets are already queued on the HWDGE ring and the runtime
            # drains the queues before surfacing completion (verified by
            # back-to-back executions in test.py).
            for g in range(g_total):
                vd = v[P * 3 * goffs[g] : P * 3 * goffs[g + 1]].rearrange(
                    "(p q) -> p q", p=P
                )
                sync.dma_start(
                    out=itb.ap()[:, 3 * goffs[g] : 3 * goffs[g + 1]], in_=vd
                ).then_inc(s_in[g], 16)
            for t in range(t_total):
                f = f_list[t]
                if t == 0:
                    # first tile: deg2 as soon as squares+cross land so the
                    # ring has output bytes ~1us earlier than a merged store
                    sync.wait_ge(s_q, t + 1)
                    sync.wait_ge(s_x, t + 1)
                    sync.dma_start(
                        out=od_flat(t)[:, 0 : 6 * f],
                        in_=ot_flat(t)[:, 0 : 6 * f],
                    ).then_inc(s_os, 16)
                    sync.wait_ge(s_d3, t + 1)
                    sync.dma_start(
                        out=od_flat(t)[:, 6 * f : KD * f],
                        in_=ot_flat(t)[:, 6 * f : KD * f],
                    ).then_inc(s_os, 16)
                elif t <= 2:
                    # ramp tiles: fine 3-way split so deg3 bytes hit the
                    # still-thin ring queue as early as possible
                    sync.wait_ge(s_q, t + 1)
                    sync.wait_ge(s_x, t + 1)
                    sync.dma_start(
                        out=od_flat(t)[:, 0 : 6 * f], in_=ot_flat(t)[:, 0 : 6 * f]
                    ).then_inc(s_os, 16)
                    sync.wait_ge(s_dA, t + 1)
                    sync.dma_start(
                        out=od_flat(t)[:, 6 * f : 12 * f],
                        in_=ot_flat(t)[:, 6 * f : 12 * f],
                    ).then_inc(s_os, 16)
                    sync.wait_ge(s_d3, t + 1)
                    sync.dma_start(
                        out=od_flat(t)[:, 12 * f : KD * f],
                        in_=ot_flat(t)[:, 12 * f : KD * f],
                    ).then_inc(s_os, 16)
                else:
                    # steady-state tiles: ring is backlogged, so chunk
                    # readiness is irrelevant - one merged [0:16] store per
                    # tile halves the descriptor count (and the straggling
                    # descriptor-duty SDMA engine 15 runs big rows at full
                    # rate, small/mid rows at ~60%)
                    sync.wait_ge(s_q, t + 1)
                    sync.wait_ge(s_d3, t + 1)
                    sync.dma_start(
                        out=od_flat(t), in_=ot_flat(t)
                    ).then_inc(s_os, 16)

        @block.scalar
        def _(scalar):
            # ACT: one 3-wide square per tile (planes 0:3 = x2,y2,z2), fully
            # decoupled from the DMA rings so DVE's s_q wait never stalls.
            # Warmup square FIRST, before the input wait: insert_act_table_
            # loads places the 1.3us ACT_TABLE_LOAD before the first
            # ACTIVATE on the path, so this hoists it off the tile-0
            # critical path into the input-DMA dead time.
            nc.scalar.square(warm.ap()[:, 0:1], warm.ap()[:, 1:2])
            seen = set()
            for t in range(t_total):
                g = tile_group[t]
                if g not in seen:
                    seen.add(g)
                    scalar.wait_ge(s_in[g], 16)
                nc.scalar.square(plane(t, 0, 3), in3(t, 0, 3)).then_inc(s_q, 1)

        @block.vector
        def _(vector):
            seen = set()
            for t in range(t_total):
                g = tile_group[t]
                if g not in seen:
                    seen.add(g)
                    vector.wait_ge(s_in[g], 16)
                # deg2 cross: xy,xz -> planes 3:5; yz -> plane 5
                nc.vector.tensor_mul(plane(t, 3, 2), bcast(t, 0, 2), in3(t, 1, 2))
                nc.vector.tensor_mul(plane(t, 5), in3(t, 1), in3(t, 2)).then_inc(
                    s_x, 1
                )
                # deg3: x*deg2[0:6] -> 6:12 (x3,xy2,xz2,x2y,x2z,xyz);
                # y*deg2[1:3] -> 12:14 (y3,yz2); z*deg2[1:3] -> 14:16
                # (y2z,z3). Needs ACT's squares (planes 0:3).
                vector.wait_ge(s_q, t + 1)
                nc.vector.tensor_mul(
                    plane(t, 6, 6), bcast(t, 0, 6), plane(t, 0, 6)
                ).then_inc(s_dA, 1)
                nc.vector.tensor_mul(plane(t, 12, 2), bcast(t, 1, 2), plane(t, 1, 2))
                nc.vector.tensor_mul(
                    plane(t, 14, 2), bcast(t, 2, 2), plane(t, 1, 2)
                ).then_inc(s_d3, 1)

    return nc


_CACHE: dict[str, object] = {}


def _get_nc() -> bass.Bass:
    if "nc" not in _CACHE:
        nc = bass.Bass(enable_partition_id=False, monotonic_sem_count=0)
        build(nc, F_LIST, G_LIST, TILE_GROUP)
        _CACHE["nc"] = nc
    return _CACHE["nc"]  # type: ignore[return-value]


def run_spmd(in_maps, trace=False, **kw):
    return run_bass_kernel_spmd(
        _get_nc(), in_maps, core_ids=list(range(N_CORES)), trace=trace, **kw
    )


_GOFFS = np.concatenate([[0], np.cumsum(G_LIST)]).astype(int)
_OFFS = np.concatenate([[0], np.cumsum(F_LIST)]).astype(int)


def to_planar(shard: np.ndarray) -> np.ndarray:
    """[n_pad, 3] f32 -> flat [P*3*F_TOTAL] per-GROUP planar blocks."""
    arr = shard.reshape(P, F_TOTAL, 3)
    parts = [
        np.ascontiguousarray(
            arr[:, _GOFFS[g] : _GOFFS[g + 1], :].transpose(0, 2, 1)
        ).reshape(-1)
        for g in range(len(G_LIST))
    ]
    return np.concatenate(parts)


# descale in REFERENCE column order: deg2 cols 0-5, deg3 cols 6-15
_PLANE_DESCALE_REF = np.array([DESCALE2] * 6 + [DESCALE3] * 10, dtype=np.float32)
_COL_TO_PLANE = np.asarray(COL_TO_PLANE)


def from_planar(dev_out: np.ndarray) -> np.ndarray:
    """flat [P*16*F_TOTAL] (any dtype) -> [n_pad, 16] f32 in ref order."""
    arr = np.asarray(dev_out, dtype=np.float32).reshape(-1)
    out = np.empty((P, F_TOTAL, KD), dtype=np.float32)
    for t in range(len(F_LIST)):
        f = F_LIST[t]
        pos = P * KD * _OFFS[t]
        blk = arr[pos : pos + P * KD * f].reshape(P, KD, f).transpose(0, 2, 1)
        out[:, _OFFS[t] : _OFFS[t + 1], :] = blk[:, :, _COL_TO_PLANE]
    out *= _PLANE_DESCALE_REF
    return out.reshape(N_PAD, KD)


def make_in_maps(vectors: np.ndarray):
    vectors = np.ascontiguousarray(np.asarray(vectors, dtype=np.float32))
    assert vectors.shape == (N_TOTAL, 3)
    shards = vectors.reshape(N_CORES, N_CORE, 3)
    in_maps = []
    for i in range(N_CORES):
        buf = np.zeros((N_PAD, 3), dtype=np.float32)
        buf[:N_CORE] = shards[i]
        in_maps.append(
            {"vectors": (to_planar(buf) * np.float32(SCALE_IN)).astype(np.float16)}
        )
    return in_maps


def kernel(vectors: np.ndarray) -> np.ndarray:
    vec32 = np.ascontiguousarray(np.asarray(vectors, dtype=np.float32))
    res = run_spmd(make_in_maps(vec32))
    out = np.empty((N_TOTAL, K), dtype=np.float32)
    out[:, 0] = 1.0
    out[:, 1:4] = vec32  # degree-1 monomials are the input, exactly
    for i in range(N_CORES):
        out[i * N_CORE : (i + 1) * N_CORE, 4:] = from_planar(res.results[i]["out"])[
            :N_CORE
        ]
    return out

